# revision 32
# baseline (speedup 1.0000x reference)
"""Trainium2 Bass kernel for nn_CrossAttentionFusion (V=3, B=8192, H=2048, NH=16).

Strategy (v3 — fused weights + scheduling):
  - Data-parallel: batch B=8192 split across 8 NeuronCores (1024 each),
    processed in 2 chunks of Bc=512 columns so every intermediate stays
    SBUF-resident (no DRAM spills).
  - Weight fusion on host (constant folding of back-to-back Linears):
        q2  = (Wiq Wq) x_i                          1 pass
        dk2 = (Wik Wk_s0) x_s0 - (Wik Wk_s1) x_s1   2 passes (PSUM-accum,
              second weight negated on host, biases folded)
        v2j = (Wiv Wv_sj) x_sj                      2 passes
        out += (Wout_i Wo_i) ctx                    1 pass
    -> 18 HxH matmul passes/core vs 27 unfused.
  - Softmax over V-1=2 key views == sigmoid:
        a0 = sigmoid((q2 . dk2)/sqrt(HD)) per head (head == 128-row tile)
        ctx = v21 + a0*(v20 - v21)
  - All matmuls bf16 (same PE rate as fp32r on TRN2, half the DMA bytes);
    PSUM accumulation fp32; residual/LN path fp32.
  - Scheduling: one wide DMA per x view; next-chunk x prefetched during
    the previous chunk; LN stats colsums interleaved into the last
    output pass; residual added during the i=0 output eviction.
"""

import math

import numpy as np

V = 3
B = 8192
H = 2048
NH = 16
HD = H // NH
EPS = 1e-5
N_CORES = 8
BPC = B // N_CORES         # 1024 batch columns per core
NCH = 2                    # chunks per core
BC = BPC // NCH            # 512 batch columns per chunk
NT = H // 128              # 16 h-tiles (== heads)
SCALE = 1.0 / math.sqrt(HD)
NP = 6 * V                 # weight passes: per i: q2,k20,k21,v20,v21,uo

# others[i] = sources of keys/values for query view i
S0 = [1, 0, 0]
S1 = [2, 2, 1]

_CACHE = {}


def _build_program(skip_gb):
    import concourse.bass as bass
    import concourse.bacc as bacc
    import concourse.tile as tile
    import concourse.mybir as mybir

    f32 = mybir.dt.float32
    f32r = mybir.dt.float32r
    bf16 = mybir.dt.bfloat16
    AF = mybir.ActivationFunctionType
    ALU = mybir.AluOpType

    nc = bacc.Bacc("TRN2", target_bir_lowering=False, debug=False,
                   num_devices=N_CORES)

    # ---- External I/O ----
    # per (chunk, view): one contiguous [128, NT*BC] bf16 block
    xbf = nc.dram_tensor("xbf", [NCH, V, 128, NT * BC], bf16,
                         kind="ExternalInput").ap()
    # views[0]^T + (Wout_blk0 @ bo0 + bout) — residual, pre-biased
    x0a = nc.dram_tensor("x0a", [NT, 128, BPC], f32r,
                         kind="ExternalInput").ap()
    # fused lhsT weights, tiled: [pass, gg, hp, ht*256+gc]
    wall = nc.dram_tensor("wall", [NP, 8, 128, NT * 256], bf16,
                          kind="ExternalInput").ap()
    # bias pack: 0-2 bq2, 3-5 bdk, 6-8 bv20, 9-11 bv21, 12-14 bwo(0 unused),
    # 15 gamma, 16 beta; partition-major so the load is one contiguous
    # line per partition
    bpk = nc.dram_tensor("bpk", [128, 17, NT], f32, kind="ExternalInput").ap()
    onesb = nc.dram_tensor("onesb", [128, 2], bf16, kind="ExternalInput").ap()
    onesr = nc.dram_tensor("onesr", [128, 2], f32r, kind="ExternalInput").ap()
    out = nc.dram_tensor("out", [NT, 128, BPC], f32, kind="ExternalOutput").ap()

    # ---- DRAM scratch ----
    a0d = nc.dram_tensor("a0d", [V, NT, BPC], bf16).ap()
    xacc = nc.dram_tensor("xacc", [NT, 128, BPC], f32r).ap()
    ab_d = nc.dram_tensor("ab_d", [NCH, 2, BC], f32).ap()

    with tile.TileContext(nc) as tc:
        ctxs = []

        def pool(name, bufs, space=None):
            kw = dict(name=name, bufs=bufs)
            if space:
                kw["space"] = space
            p = tc.tile_pool(**kw)
            ctxs.append(p)
            return p.__enter__()

        cst = pool("cst", 1)
        xp = pool("xp", 2)        # 3 tags x 16KB x 2          = 96KB
        wp = pool("wp", 2)        # 1 tag  x 8KB x 2           = 16KB
        imq = pool("imq", 1)      # 16 tags x 1KB              = 16KB
        imc = pool("imc", 1)      # 16 tags x 1KB              = 16KB
        dks = pool("dks", 4)      # 1 tag x 1KB x 4            = 4KB
        v2s = pool("v2s", 4)      # 1 tag x 1KB x 4            = 4KB
        bcp = pool("bcp", 3)      # 1 tag x 1KB x 3            = 3KB
        a0p = pool("a0p", 4)      # 1 tag x 1KB x 4            = 4KB
        evp = pool("evp", 4)      # 1 tag x 2KB x 4            = 8KB
        x0p = pool("x0p", 3)      # 1 tag x 2KB x 3            = 6KB
        lnap = pool("lnap", 3)    # stats readback stream       = 6KB
        lnbp = pool("lnbp", 3)    # normalize readback stream   = 6KB
        lnsp = pool("lnsp", 2)    # squares stream              = 4KB
        lnnp = pool("lnnp", 3)    # normalized output stream    = 6KB
        stt = pool("stt", 1)      # mu/m2/msq 2KB + A/B 2KB    = 10KB
        psp = pool("psp", 1, space="PSUM")

        # constants (bias pack DMA is issued after the first x/weight
        # loads — it is first needed only at the first eviction)
        bias_sb = cst.tile([128, 17, NT], f32)
        ones_b = cst.tile([128, 1], bf16)
        nc.sync.dma_start(ones_b[:], onesb[:, 0:1])
        ones_r = cst.tile([128, 1], f32r)
        nc.sync.dma_start(ones_r[:], onesr[:, 0:1])
        eps_t = cst.tile([1, 1], f32)
        nc.vector.memset(eps_t[:], EPS)

        def eng(k):
            return nc.vector if k % 2 == 0 else nc.gpsimd

        xt_next = {}

        def load_x(c, v, split=1):
            tl = xp.tile([128, NT * BC], bf16, tag=f"x{v}", name=f"x{v}")
            step = NT * BC // split
            for k in range(split):
                nc.sync.dma_start(tl[:, k * step:(k + 1) * step],
                                  xbf[c, v][:, k * step:(k + 1) * step])
            return tl

        for c in range(NCH):
            cs = c * BC
            xt = dict(xt_next)
            xt_next = {}

            def mm_pass(weights, movings, evict_cb, pre_cb=None, pre_w=None):
                """One (or accumulated multi-) HxH projection pass.

                movings[si]: wide x tile ([128, NT*BC], sliced per ht) or a
                list of 16 [128, BC] tiles.
                """
                n = len(weights)
                for gg in range(8):
                    pts = [psp.tile([128, BC], f32,
                                    tag=f"pp{2 * (gg % 2) + gi}",
                                    name=f"pt{gi}") for gi in range(2)]
                    if pre_cb is not None:
                        pre_cb(gg)
                    for si in range(n):
                        w = pre_w.get((si, gg)) if pre_w else None
                        if w is None:
                            w = wp.tile([128, NT * 256], bf16, tag="w",
                                        name="w")
                            nc.sync.dma_start(w[:], wall[weights[si], gg])
                        mt = movings[si]
                        for ht in range(NT):
                            base = ht * 256
                            if isinstance(mt, list):
                                rhs = mt[ht][:]
                            else:
                                rhs = mt[:, ht * BC:(ht + 1) * BC]
                            for gi in range(2):
                                nc.tensor.matmul(
                                    pts[gi][:],
                                    w[:, base + gi * 128:base + gi * 128 + 128],
                                    rhs,
                                    start=(si == 0 and ht == 0),
                                    stop=(si == n - 1 and ht == NT - 1))
                    for gi in range(2):
                        evict_cb(gg * 2 + gi, pts[gi])

            for i in range(V):
                p0 = 6 * i
                w00 = None
                if c == 0 and i == 0:
                    # critical startup path: first weight block, then the
                    # first view in halves, then everything else
                    w00 = wp.tile([128, NT * 256], bf16, tag="w", name="w")
                    nc.sync.dma_start(w00[:], wall[p0, 0])
                    xt[i] = load_x(c, i, split=2)
                    nc.sync.dma_start(bias_sb[:], bpk)

                # ---- q2 pass -> imq resident ----
                qt = [None] * NT

                def ev_q2(gt, pt):
                    q = imq.tile([128, BC], bf16, tag=f"q{gt}", name=f"q{gt}")
                    nc.scalar.activation(q[:], pt[:], AF.Identity,
                                         bias=bias_sb[:, 0 + i, gt:gt + 1])
                    qt[gt] = q

                mm_pass([p0 + 0], [xt[i]], ev_q2,
                        pre_w={(0, 0): w00} if w00 is not None else None)

                if c == 0 and i == 0:
                    xt[S0[i]] = load_x(c, S0[i])
                    xt[S1[i]] = load_x(c, S1[i])

                # ---- dk2 pass (2 weights, PSUM-accumulated) + scores ----
                def ev_dk(gt, pt):
                    dk = dks.tile([128, BC], bf16, tag="dk", name="dk")
                    nc.scalar.activation(dk[:], pt[:], AF.Identity,
                                         bias=bias_sb[:, 3 + i, gt:gt + 1])
                    eng(gt).tensor_tensor(dk[:], dk[:], qt[gt][:], ALU.mult)
                    cs_t = psp.tile([128, BC], f32, tag=f"pp{4 + gt % 2}",
                                    name="cs")
                    nc.tensor.matmul(cs_t[0:1, :], ones_b[:], dk[:],
                                     start=True, stop=True)
                    a0t = a0p.tile([1, BC], bf16, tag="a0", name="a0")
                    nc.scalar.activation(a0t[:], cs_t[0:1, :], AF.Sigmoid,
                                         scale=SCALE)
                    nc.sync.dma_start(a0d[i, gt:gt + 1, cs:cs + BC], a0t[:])

                mm_pass([p0 + 1, p0 + 2], [xt[S0[i]], xt[S1[i]]], ev_dk)

                # prefetch next chunk's x views (overlaps v/uo passes)
                if i == V - 1 and c + 1 < NCH:
                    for v in range(V):
                        xt_next[v] = load_x(c + 1, v)

                # ---- v20 pass -> imc resident ----
                ct = [None] * NT

                def ev_v20(gt, pt):
                    t_ = imc.tile([128, BC], bf16, tag=f"c{gt}", name=f"c{gt}")
                    nc.scalar.activation(t_[:], pt[:], AF.Identity,
                                         bias=bias_sb[:, 6 + i, gt:gt + 1])
                    ct[gt] = t_

                mm_pass([p0 + 3], [xt[S0[i]]], ev_v20)

                # ---- v21 pass + fused ctx = v21 + a0*(v20-v21) ----
                bct = [None] * NT

                def pre_v21(gg):
                    for gi in range(2):
                        gt = gg * 2 + gi
                        b = bcp.tile([128, BC], bf16, tag="bc", name="bc")
                        src = a0d[i, gt, cs:cs + BC]
                        a0b = bass.AP(tensor=src.tensor, offset=src.offset,
                                      ap=[[0, 128], [1, BC]])
                        nc.sync.dma_start(b[:], a0b)
                        bct[gt] = b

                def ev_v21(gt, pt):
                    v2 = v2s.tile([128, BC], bf16, tag="v2", name="v2")
                    nc.scalar.activation(v2[:], pt[:], AF.Identity,
                                         bias=bias_sb[:, 9 + i, gt:gt + 1])
                    e = eng(gt)
                    e.tensor_tensor(ct[gt][:], ct[gt][:], v2[:], ALU.subtract)
                    e.tensor_tensor(ct[gt][:], ct[gt][:], bct[gt][:], ALU.mult)
                    e.tensor_tensor(ct[gt][:], ct[gt][:], v2[:], ALU.add)

                mm_pass([p0 + 4], [xt[S1[i]]], ev_v21, pre_cb=pre_v21)

                # ---- output pass -> xacc (+residual for i=0) ----
                x0t = [None] * NT

                def pre_uo(gg):
                    if i != 0:
                        return
                    for gi in range(2):
                        gt = gg * 2 + gi
                        t_ = x0p.tile([128, BC], f32r, tag="x0", name="x0")
                        nc.sync.dma_start(t_[:], x0a[gt, :, cs:cs + BC])
                        x0t[gt] = t_

                def ev_uo(gt, pt):
                    ev = evp.tile([128, BC], f32r, tag="ev", name="ev")
                    xsl = xacc[gt, :, cs:cs + BC]
                    if i == 0:
                        nc.vector.tensor_tensor(ev[:], pt[:].bitcast(f32r),
                                                x0t[gt][:], ALU.add)
                        nc.sync.dma_start(xsl, ev[:])
                    else:
                        nc.scalar.activation(ev[:], pt[:], AF.Identity,
                                             bias=bias_sb[:, 12 + i,
                                                          gt:gt + 1])
                        nc.gpsimd.dma_start(xsl, ev[:], accum_op=ALU.add)

                mm_pass([p0 + 5], [ct], ev_uo, pre_cb=pre_uo)

            # ===== LayerNorm (feature-dim stats via PE colsums) =====
            # Readbacks stream with prefetch depth 5; by the time the PE
            # reaches the colsums, early tiles' accumulates finished long
            # ago, so the pipeline doesn't stall the tensor queue.
            psx = psp.tile([128, BC], f32, tag="pp6", name="psx")
            psxx = psp.tile([128, BC], f32, tag="pp7", name="psxx")
            for t in range(NT):
                xa = lnap.tile([128, BC], f32r, tag="lna", name="lna")
                # scalar-queue DMA: keeps the sync queue free for the next
                # chunk's weight fetches (no head-of-line blocking)
                nc.scalar.dma_start(xa[:], xacc[t, :, cs:cs + BC])
                sq = lnsp.tile([128, BC], f32r, tag="lnsq", name="lnsq")
                eng(t).tensor_tensor(sq[:], xa[:], xa[:], ALU.mult)
                nc.tensor.matmul(psx[0:1, :], ones_r[:], xa[:],
                                 start=(t == 0), stop=(t == NT - 1))
                nc.tensor.matmul(psxx[0:1, :], ones_r[:], sq[:],
                                 start=(t == 0), stop=(t == NT - 1))
            mu = stt.tile([1, BC], f32, tag="mu", name="mu")
            m2 = stt.tile([1, BC], f32, tag="m2", name="m2")
            msq = stt.tile([1, BC], f32, tag="msq", name="msq")
            nc.scalar.activation(mu[:], psx[0:1, :], AF.Copy, scale=1.0 / H)
            nc.scalar.activation(m2[:], psxx[0:1, :], AF.Copy, scale=1.0 / H)
            nc.vector.tensor_tensor(msq[:], mu[:], mu[:], ALU.mult)
            nc.vector.tensor_tensor(m2[:], m2[:], msq[:], ALU.subtract)
            nc.scalar.activation(m2[:], m2[:], AF.Sqrt, bias=eps_t[:])
            nc.vector.reciprocal(m2[:], m2[:])             # rstd
            nc.vector.tensor_tensor(mu[:], mu[:], m2[:], ALU.mult)
            nc.scalar.activation(mu[:], mu[:], AF.Copy, scale=-1.0)
            # broadcast rstd / -mu*rstd along partitions via DMA round trip
            # (keeps the in-order PE queue free of the mu/var dependency)
            nc.scalar.dma_start(ab_d[c, 0:1, :], m2[:])
            nc.scalar.dma_start(ab_d[c, 1:2, :], mu[:])
            A_sb = stt.tile([128, BC], f32, tag="A", name="A")
            B_sb = stt.tile([128, BC], f32, tag="B", name="B")
            for k, dst in ((0, A_sb), (1, B_sb)):
                src = ab_d[c, k]
                bc_ap = bass.AP(tensor=src.tensor, offset=src.offset,
                                ap=[[0, 128], [1, BC]])
                nc.scalar.dma_start(dst[:], bc_ap)
            for t in range(NT):
                e = eng(t)
                de = nc.scalar if t % 2 == 0 else nc.gpsimd
                xa = lnbp.tile([128, BC], f32r, tag="lnb", name="lnb")
                de.dma_start(xa[:], xacc[t, :, cs:cs + BC])
                n1 = lnnp.tile([128, BC], f32, tag="lnn", name="lnn")
                e.tensor_tensor(n1[:], xa[:].bitcast(f32), A_sb[:], ALU.mult)
                e.tensor_tensor(n1[:], n1[:], B_sb[:], ALU.add)
                if not skip_gb:
                    e.tensor_scalar(
                        out=n1[:], in0=n1[:],
                        scalar1=bias_sb[:, 15, t:t + 1],
                        scalar2=bias_sb[:, 16, t:t + 1],
                        op0=ALU.mult, op1=ALU.add)
                de.dma_start(out[t, :, cs:cs + BC], n1[:])

        for p in reversed(ctxs):
            p.__exit__(None, None, None)

    nc.compile()
    return nc


def _prep_host(inputs):
    """Fuse weight pairs (constant folding) + pack layouts on host."""
    import ml_dtypes
    bf = ml_dtypes.bfloat16
    f32 = np.float32

    views = np.asarray(inputs["views"], f32)
    g = {k: np.asarray(inputs[k], f32) for k in
         ("Wq", "bq", "Wk", "bk", "Wv", "bv", "Wiq", "biq", "Wik", "bik",
          "Wiv", "biv", "Wo", "bo", "Wout", "bout", "gamma", "beta")}

    def pack(lhsT):
        # [H_in, H_out] -> [8 gg, 128 hp, (16 ht)*256 gc]
        t = lhsT.reshape(NT, 128, 8, 256).transpose(2, 1, 0, 3)
        return np.ascontiguousarray(t.reshape(8, 128, NT * 256)).astype(bf)

    wlist = []
    bpk = np.zeros((17, 128, NT), f32)

    def bcol(vec):
        return vec.reshape(NT, 128).T

    Wout_blk = [g["Wout"][:, i * H:(i + 1) * H] for i in range(V)]
    for i in range(V):
        wlist.append(pack((g["Wiq"][i] @ g["Wq"][i]).T))
        wlist.append(pack((g["Wik"][i] @ g["Wk"][S0[i]]).T))
        wlist.append(pack(-(g["Wik"][i] @ g["Wk"][S1[i]]).T))
        wlist.append(pack((g["Wiv"][i] @ g["Wv"][S0[i]]).T))
        wlist.append(pack((g["Wiv"][i] @ g["Wv"][S1[i]]).T))
        wlist.append(pack((Wout_blk[i] @ g["Wo"][i]).T))
        bpk[0 + i] = bcol(g["Wiq"][i] @ g["bq"][i] + g["biq"][i])
        bpk[3 + i] = bcol(g["Wik"][i] @ (g["bk"][S0[i]] - g["bk"][S1[i]]))
        bpk[6 + i] = bcol(g["Wiv"][i] @ g["bv"][S0[i]] + g["biv"][i])
        bpk[9 + i] = bcol(g["Wiv"][i] @ g["bv"][S1[i]] + g["biv"][i])
        if i > 0:
            bpk[12 + i] = bcol(Wout_blk[i] @ g["bo"][i])
    bpk[15] = bcol(g["gamma"])
    bpk[16] = bcol(g["beta"])

    shared = {
        "wall": np.ascontiguousarray(np.stack(wlist)),
        "bpk": np.ascontiguousarray(bpk.transpose(1, 0, 2)),
        "onesb": np.ones((128, 2), bf),
        "onesr": np.ones((128, 2), f32),
    }

    # residual (views[0]) pre-biased with the i=0 output-pass bias
    res_bias = (Wout_blk[0] @ g["bo"][0] + g["bout"]).astype(f32)

    percore = []
    for cc in range(N_CORES):
        sl = views[:, cc * BPC:(cc + 1) * BPC, :]          # [V, BPC, H]
        xfm = np.ascontiguousarray(sl.transpose(0, 2, 1))  # [V, H, BPC]
        x4 = xfm.reshape(V, NT, 128, BPC)
        # [chunk, view, partition, ht*BC]
        xc = np.empty((NCH, V, 128, NT * BC), f32)
        for ch in range(NCH):
            blk = x4[:, :, :, ch * BC:(ch + 1) * BC]       # [V, NT, 128, BC]
            xc[ch] = blk.transpose(0, 2, 1, 3).reshape(V, 128, NT * BC)
        x0 = (xfm[0] + res_bias[:, None]).reshape(NT, 128, BPC)
        percore.append({"xbf": xc.astype(bf),
                        "x0a": np.ascontiguousarray(x0)})
    return shared, percore


def kernel(**inputs):
    from concourse.bass_utils import run_bass_kernel_spmd

    trace = bool(_CACHE.get("trace", False))
    skip_gb = bool(np.all(np.asarray(inputs["gamma"]) == 1.0)
                   and np.all(np.asarray(inputs["beta"]) == 0.0))
    if _CACHE.get("nc_key") != skip_gb:
        _CACHE["nc"] = _build_program(skip_gb)
        _CACHE["nc_key"] = skip_gb
    nc = _CACHE["nc"]

    shared, percore = _prep_host(inputs)
    in_maps = []
    for cc in range(N_CORES):
        m = dict(shared)
        m.update(percore[cc])
        in_maps.append(m)

    res = run_bass_kernel_spmd(nc, in_maps, core_ids=list(range(N_CORES)),
                               trace=trace)
    _CACHE["last_result"] = res

    outp = np.empty((B, H), np.float32)
    for cc in range(N_CORES):
        o = np.asarray(res.results[cc]["out"], np.float32).reshape(H, BPC)
        outp[cc * BPC:(cc + 1) * BPC, :] = o.T
    return outp
